# revision 1
# baseline (speedup 1.0000x reference)
"""Causal self-attention (B=2, T=2048, D=768, H=12) on 8 TRN2 cores.

Sharding: core r handles batch b=r//4 and head-group g=r%4 (3 heads).
  - qkv projection: tensor-parallel slice of W_qkv (this core's 3 heads).
  - attention: fully local per (b, head).
  - reshard: one 8-core AllToAll moves pre-projection attention outputs
    (O^T, feature-major) so that core r ends up with the full 768
    attention features for token block r*256:(r+1)*256 of BOTH batches.
  - proj: local matmul over the full contraction dim -> y[b, t_block, :].
Host side only shards/transposes inputs and concatenates outputs.

Device-side layout notes:
  - All matmuls contract over the SBUF partition dim. x and weights are
    host-pre-transposed so in-features land on partitions.
  - Attention computes S^T[j,i] = k_j . q_i (keys on partitions). Softmax
    runs without max-subtraction (logits ~ N(0,1), |S|<~7 here), so
    exp(S/8) is taken directly and the denominator is produced by an
    extra ones-row appended to V in the O^T = V_aug^T E matmul.
  - Normalization: reciprocal of the denominator row, partition-broadcast,
    multiply. O^T [64, T] per head is already the layout the proj needs.
  - fp16 matmul inputs (11-bit mantissa ~= the PE's single-pass fp32r
    mode, but true 1 cycle/row + fast weight loads); fp32 PSUM accumulate.
"""

import numpy as np

import concourse.bass as bass
import concourse.bacc as bacc
import concourse.mybir as mybir
import concourse.tile as tile
from concourse.bass_utils import run_bass_kernel_spmd

F32 = mybir.dt.float32
F16 = mybir.dt.float16

B, T, D = 2, 2048, 768
H, DH = 12, 64
NCORES = 8
HPC = H // 4          # heads per core = 3
QK = HPC * DH         # 192 rows of q (or k) per core
KC = D // 128         # 6 contraction chunks
TBLK = T // NCORES    # 256 tokens of proj output per core

EXP_SCALE = 1.0 / np.sqrt(DH) # 0.125


def _emit(tc, aps):
    nc = tc.nc
    xT, wqkT, wvT, wpT, triu, y = (
        aps["xT"], aps["wqkT"], aps["wvT"], aps["wpT"], aps["triu"], aps["y"])

    ctx_pools = {}

    def pool(name, bufs, space="SBUF"):
        p = tc.tile_pool(name=name, bufs=bufs, space=space)
        ctx_pools[name] = p
        return p.__enter__()

    def close_pool(name):
        ctx_pools.pop(name).__exit__(None, None, None)

    consts = pool("consts", 1)
    qk_sb = pool("qk_sb", 1)
    v_sb = pool("v_sb", 1)
    ot_sb = pool("ot_sb", 1)
    work = pool("work", 3)
    norm = pool("norm", 2)
    dram = pool("dram", 1, space="DRAM")
    # opened last so it can be closed (stack-top) after qkv to make room
    xw = pool("xw", 1)
    ps_qkv = pool("ps_qkv", 1, space="PSUM")

    # ---- loads: x first so qkv matmuls can start ASAP; wp last ----
    triu_sb = consts.tile([128, 128], F16, tag="triu", name="triu")
    nc.sync.dma_start(triu_sb[:], triu[:, :])

    xT_sb = [xw.tile([128, T], F16, tag=f"xT{k}", name=f"xT{k}") for k in range(KC)]
    wqk_sb = [consts.tile([128, 2 * QK], F16, tag=f"wqk{k}", name=f"wqk{k}") for k in range(KC)]
    wv_sb = [consts.tile([128, QK], F16, tag=f"wv{k}", name=f"wv{k}") for k in range(KC)]
    # x chunks split across three DMA queues so the first qkv K-loop can
    # start as early as possible; weights go through gpsimd's queue
    x_engs = [nc.sync, nc.scalar, nc.sync]
    for k in range(KC):
        x_engs[k % 3].dma_start(xT_sb[k][:], xT[k * 128:(k + 1) * 128, :])
        nc.gpsimd.dma_start(wqk_sb[k][:], wqkT[k * 128:(k + 1) * 128, :])
        nc.gpsimd.dma_start(wv_sb[k][:], wvT[k * 128:(k + 1) * 128, :])

    # proj weights last (needed only at the end; loads overlap attention),
    # regrouped to the exchanged chunk boundaries: per head-group g of the
    # b-group, A rows = [192g, 192g+128) (heads 3g,3g+1), B = [+128, +64)
    wpA_sb = [consts.tile([128, D], F16, tag=f"wpA{k}", name=f"wpA{k}") for k in range(4)]
    wpB_sb = [consts.tile([64, D], F16, tag=f"wpB{k}", name=f"wpB{k}") for k in range(4)]
    for k in range(4):
        nc.gpsimd.dma_start(wpA_sb[k][:], wpT[192 * k:192 * k + 128, :])
        nc.gpsimd.dma_start(wpB_sb[k][:], wpT[192 * k + 128:192 * k + 192, :])

    # ---- qkv projections ----
    # heads 0/1 packed into [128, T] tiles (rows 0-63 / 64-127) so their S
    # matmuls can use PE row tile_position 0 / 64 concurrently
    qTp = qk_sb.tile([128, T], F16, tag="qTp", name="qTp")
    kTp = qk_sb.tile([128, T], F16, tag="kTp", name="kTp")
    qT2 = qk_sb.tile([64, T], F16, tag="qT2", name="qT2")
    kT2 = qk_sb.tile([64, T], F16, tag="kT2", name="kT2")
    qT = [qTp[0:64], qTp[64:128], qT2[:, :]]
    kT = [kTp[0:64], kTp[64:128], kT2[:, :]]

    VW = 65
    v_aug = [v_sb.tile([128, (T // 128) * VW], F16, tag=f"v{h}", name=f"v{h}") for h in range(HPC)]

    def emit_qkv_chunk(n):
        ns = slice(n * 512, (n + 1) * 512)
        for m in range(3):
            ps = ps_qkv.tile([128, 512], F32, tag="qkps", name="qkps")
            for k in range(KC):
                nc.tensor.matmul(
                    ps[:],
                    wqk_sb[k][:, m * 128:(m + 1) * 128],
                    xT_sb[k][:, ns],
                    start=(k == 0), stop=(k == KC - 1))
            if m == 0:
                nc.vector.tensor_copy(qTp[:, ns], ps[:])
            elif m == 1:
                nc.vector.tensor_copy(qT2[:, ns], ps[0:64, :])
                nc.vector.tensor_copy(kTp[0:64, ns], ps[64:128, :])
            else:
                nc.vector.tensor_copy(kTp[64:128, ns], ps[0:64, :])
                nc.vector.tensor_copy(kT2[:, ns], ps[64:128, :])
        for tt in range(n * 4, n * 4 + 4):
            ps = ps_qkv.tile([128, QK], F32, tag="vps", name="vps")
            for k in range(KC):
                nc.tensor.matmul(
                    ps[:],
                    xT_sb[k][:, tt * 128:(tt + 1) * 128],
                    wv_sb[k][:],
                    start=(k == 0), stop=(k == KC - 1))
            for h in range(HPC):
                nc.vector.tensor_copy(
                    v_aug[h][:, tt * VW:tt * VW + 64], ps[:, h * 64:(h + 1) * 64])
                # ones column: triu col 127 is all-ones
                nc.vector.tensor_copy(
                    v_aug[h][:, tt * VW + 64:tt * VW + 65], triu_sb[:, 127:128])

    ps_qkv_ = None  # placeholder (pool opened above)
    ps_s = pool("ps_s", 2, space="PSUM")
    ps_o = pool("ps_o", 2, space="PSUM")
    proj_sb = pool("proj_sb", 1)

    # ---- attention ----
    # S^T[j, i] = k_j . q_i ; E = exp(S^T/8) causal-masked; O_un[65, i] =
    # sum_j V_aug[j, :]^T E[j, i] (row 64 = softmax denominator).
    # Heads 0 and 1 are processed jointly: their S matmuls go to disjoint
    # PE row groups via tile_position and adjacent issue -> they run
    # concurrently, and share one [128,1024] PSUM strip so a single
    # ACTIVATE exponentiates both. Head 2 pairs consecutive j-tiles in the
    # same way. Keeping ACT per-tile cost below PE keeps the PE dense.
    OT_un = [ot_sb.tile([64, T], F16, tag=f"OTu{h}", name=f"OTu{h}") for h in range(HPC)]
    OT = [ot_sb.tile([64, T], F16, tag=f"OT{h}", name=f"OT{h}") for h in range(HPC)]
    den_t = [ot_sb.tile([128, 512], F32, tag=f"dent{j}", name=f"dent{j}") for j in range(4)]
    rec_t = [ot_sb.tile([128, 512], F32, tag=f"rect{j}", name=f"rect{j}") for j in range(4)]
    for j in range(4):
        nc.vector.memset(den_t[j][:], 1.0)
    # phase-split exchange: heads 0/1 (128 rows/shard) overlap head 2's
    # attention; head 2 (64 rows/shard) is the only exposed transfer.
    a2a1_in = dram.tile([NCORES * 128, TBLK], F16, tag="a2a1_in", name="a2a1_in")
    a2a1_out = dram.tile([NCORES * 128, TBLK], F16, tag="a2a1_out", name="a2a1_out")
    a2a2_in = dram.tile([NCORES * 64, TBLK], F16, tag="a2a2_in", name="a2a2_in")
    a2a2_out = dram.tile([NCORES * 64, TBLK], F16, tag="a2a2_out", name="a2a2_out")

    def finish_block(h, bi, o_ps):
        # stash unnormalized output + denominator (normalized per-bi after
        # all three heads' blocks land; see finish_bi)
        nc.vector.tensor_copy(OT_un[h][:, bi * 512:(bi + 1) * 512], o_ps[0:64, :])
        nc.vector.tensor_copy(
            den_t[bi][32 * h:32 * h + 1, :], o_ps[64:65, :])

    def finish_bi01(bi):
        nc.vector.reciprocal(rec_t[bi][:], den_t[bi][:])
        for h in range(2):
            iblk = slice(bi * 512, (bi + 1) * 512)
            stage = norm.tile([1, 512], F32, tag="stage", name="stage")
            nc.vector.tensor_copy(stage[:], rec_t[bi][32 * h:32 * h + 1, :])
            rb = norm.tile([64, 512], F32, tag="rb", name="rb")
            nc.gpsimd.partition_broadcast(rb[:], stage[:])
            nc.vector.tensor_mul(OT[h][:, iblk], OT_un[h][:, iblk], rb[:])
            for s in (2 * bi, 2 * bi + 1):
                nc.sync.dma_start(
                    a2a1_in[s * 128 + h * 64: s * 128 + (h + 1) * 64, :],
                    OT[h][:, s * TBLK:(s + 1) * TBLK])

    def finish_bi2(bi):
        iblk = slice(bi * 512, (bi + 1) * 512)
        stage = norm.tile([1, 512], F32, tag="stage", name="stage")
        nc.vector.reciprocal(stage[:], den_t[bi][64:65, :])
        rb = norm.tile([64, 512], F32, tag="rb", name="rb")
        nc.gpsimd.partition_broadcast(rb[:], stage[:])
        nc.vector.tensor_mul(OT[2][:, iblk], OT_un[2][:, iblk], rb[:])
        for s in (2 * bi, 2 * bi + 1):
            nc.sync.dma_start(
                a2a2_in[s * 64: (s + 1) * 64, :],
                OT[2][:, s * TBLK:(s + 1) * TBLK])

    def emit_av(h, o_ps, e_sb, ecol, tj, lo, ntj):
        nc.tensor.matmul(
            o_ps[:, lo:],
            v_aug[h][:, tj * VW:(tj + 1) * VW],
            e_sb[:, ecol + lo:ecol + 512],
            start=(tj == 0), stop=(tj == ntj - 1))

    def mask_diag(e_sb, ecol, lo):
        dsl = slice(ecol + lo, ecol + lo + 128)
        nc.vector.tensor_mul(e_sb[:, dsl], e_sb[:, dsl], triu_sb[:])

    # all three heads per query block: heads 0/1 jointly (concurrent S via
    # PE row tile_position), head 2 with paired j-tiles.
    # phase 1: qkv chunks + heads 0/1 jointly (concurrent S via row groups)
    for bi in range(T // 512):
        emit_qkv_chunk(bi)
        o01 = {}
        o01[0] = ps_o.tile([65, 512], F32, tag="o", name="o_a")
        o01[1] = ps_o.tile([65, 512], F32, tag="o", name="o_b")
        ntj = 4 * bi + 4
        for tj in range(ntj):
            dtile = tj - 4 * bi
            lo = max(dtile, 0) * 128
            s_ps = ps_s.tile([128, 1024], F32, tag="s", name="s")
            e_sb = work.tile([128, 1024], F16, tag="e", name="e")
            nc.tensor.matmul(
                s_ps[:, lo:512],
                kT[0][:, tj * 128:(tj + 1) * 128],
                qT[0][:, bi * 512 + lo:(bi + 1) * 512],
                start=True, stop=True, tile_position=(0, 0))
            nc.tensor.matmul(
                s_ps[:, 512 + lo:1024],
                kT[1][:, tj * 128:(tj + 1) * 128],
                qT[1][:, bi * 512 + lo:(bi + 1) * 512],
                start=True, stop=True, tile_position=(64, 0))
            if lo == 0:
                nc.scalar.activation(
                    e_sb[:], s_ps[:],
                    mybir.ActivationFunctionType.Exp, scale=EXP_SCALE)
            else:
                for half in range(2):
                    nc.scalar.activation(
                        e_sb[:, half * 512 + lo:(half + 1) * 512],
                        s_ps[:, half * 512 + lo:(half + 1) * 512],
                        mybir.ActivationFunctionType.Exp, scale=EXP_SCALE)
            for half in range(2):
                if dtile >= 0:
                    mask_diag(e_sb, half * 512, lo)
                emit_av(half, o01[half], e_sb, half * 512, tj, lo, ntj)
        for h in range(2):
            finish_block(h, bi, o01[h])
        finish_bi01(bi)

    nc.gpsimd.collective_compute(
        "AllToAll",
        mybir.AluOpType.bypass,
        replica_groups=[list(range(NCORES))],
        ins=[a2a1_in.opt()],
        outs=[a2a1_out.opt()],
    )

    # phase 2: head 2 with paired j-tiles (runs while a2a1 is in flight)
    for bi in range(T // 512):
        o_c = ps_o.tile([65, 512], F32, tag="o", name="o_c")
        ntj = 4 * bi + 4
        for tj0 in range(0, ntj, 2):
            pair = [tj for tj in (tj0, tj0 + 1) if tj < ntj]
            s_ps = ps_s.tile([128, 1024], F32, tag="s", name="s2")
            e_sb = work.tile([128, 1024], F16, tag="e", name="e2")
            los = []
            for idx, tj in enumerate(pair):
                dtile = tj - 4 * bi
                lo = max(dtile, 0) * 128
                los.append(lo)
                nc.tensor.matmul(
                    s_ps[:, idx * 512 + lo:(idx + 1) * 512],
                    kT[2][:, tj * 128:(tj + 1) * 128],
                    qT[2][:, bi * 512 + lo:(bi + 1) * 512],
                    start=True, stop=True)
            if len(pair) == 2 and los[1] == 0:
                nc.scalar.activation(
                    e_sb[:, los[0]:1024], s_ps[:, los[0]:1024],
                    mybir.ActivationFunctionType.Exp, scale=EXP_SCALE)
            else:
                for idx, tj in enumerate(pair):
                    nc.scalar.activation(
                        e_sb[:, idx * 512 + los[idx]:(idx + 1) * 512],
                        s_ps[:, idx * 512 + los[idx]:(idx + 1) * 512],
                        mybir.ActivationFunctionType.Exp, scale=EXP_SCALE)
            for idx, tj in enumerate(pair):
                if tj - 4 * bi >= 0:
                    mask_diag(e_sb, idx * 512, los[idx])
                emit_av(2, o_c, e_sb, idx * 512, tj, los[idx], ntj)
        finish_block(2, bi, o_c)
        finish_bi2(bi)

    close_pool("ps_o")
    close_pool("ps_s")
    close_pool("ps_qkv")
    ps_y = pool("ps_y", 8, space="PSUM")

    nc.gpsimd.collective_compute(
        "AllToAll",
        mybir.AluOpType.bypass,
        replica_groups=[list(range(NCORES))],
        ins=[a2a2_in.opt()],
        outs=[a2a2_out.opt()],
    )

    # exchanged O^T chunks: otfA = head-pair rows (4x[128]/batch),
    # otfB = head-2 rows (4x[64]/batch)
    otfA = proj_sb.tile([128, 8 * TBLK], F16, tag="otfA", name="otfA")
    otfB = proj_sb.tile([64, 8 * TBLK], F16, tag="otfB", name="otfB")
    for cc in range(8):
        nc.sync.dma_start(
            otfA[:, cc * TBLK:(cc + 1) * TBLK],
            a2a1_out[cc * 128:(cc + 1) * 128, :])
        nc.gpsimd.dma_start(
            otfB[:, cc * TBLK:(cc + 1) * TBLK],
            a2a2_out[cc * 64:(cc + 1) * 64, :])

    # ---- output projection: 8 live accumulation groups, c outermost so
    # matmuls start as soon as otf lands and stream without PSUM stalls ----
    y_sb = [[proj_sb.tile([128, D], F32, tag=f"y{b}_{mi}", name=f"y{b}_{mi}")
             for mi in range(TBLK // 128)] for b in range(B)]
    groups = [(b, mi, on, osz) for b in range(B) for mi in range(TBLK // 128)
              for on, osz in ((0, 512), (512, 256))]
    # A-chunk matmuls depend only on a2a1 -> they run while a2a2 is still
    # in flight; B-chunks + copies follow once otfB lands.
    ps_g = [ps_y.tile([128, osz], F32, tag="yps", name=f"yps{gi}")
            for gi, (b, mi, on, osz) in enumerate(groups)]
    for gi, (b, mi, on, osz) in enumerate(groups):
        for k4 in range(4):
            cc = b * 4 + k4
            nc.tensor.matmul(
                ps_g[gi][:],
                otfA[:, cc * TBLK + mi * 128:cc * TBLK + (mi + 1) * 128],
                wpA_sb[k4][:, on:on + osz],
                start=(k4 == 0), stop=False)
    for gi, (b, mi, on, osz) in enumerate(groups):
        for k4 in range(4):
            cc = b * 4 + k4
            nc.tensor.matmul(
                ps_g[gi][:],
                otfB[:, cc * TBLK + mi * 128:cc * TBLK + (mi + 1) * 128],
                wpB_sb[k4][:, on:on + osz],
                start=False, stop=(k4 == 3))
        nc.vector.tensor_copy(y_sb[b][mi][:, on:on + osz], ps_g[gi][:])
    for b in range(B):
        for mi in range(TBLK // 128):
            eng = nc.sync if (b + mi) % 2 == 0 else nc.gpsimd
            eng.dma_start(
                y[b, mi * 128:(mi + 1) * 128, :], y_sb[b][mi][:])

    for name in reversed(list(ctx_pools)):
        close_pool(name)


_NC_CACHE = {}


def _get_nc():
    if "nc" in _NC_CACHE:
        return _NC_CACHE["nc"]
    nc = bacc.Bacc("TRN2", num_devices=NCORES, debug=False)
    aps = {
        "xT": nc.dram_tensor("xT", [D, T], F16, kind="ExternalInput").ap(),
        "wqkT": nc.dram_tensor("wqkT", [D, 2 * QK], F16, kind="ExternalInput").ap(),
        "wvT": nc.dram_tensor("wvT", [D, QK], F16, kind="ExternalInput").ap(),
        "wpT": nc.dram_tensor("wpT", [D, D], F16, kind="ExternalInput").ap(),
        "triu": nc.dram_tensor("triu", [128, 128], F16, kind="ExternalInput").ap(),
        "y": nc.dram_tensor("y", [B, TBLK, D], F32, kind="ExternalOutput").ap(),
    }
    with tile.TileContext(nc, num_cores=NCORES) as tc:
        _emit(tc, aps)
    nc.compile()
    _NC_CACHE["nc"] = nc
    return nc


def make_in_maps(x, W_qkv, W_proj):
    triu = np.triu(np.ones((128, 128), dtype=np.float16))
    wpT = np.ascontiguousarray(W_proj.T).astype(np.float16)
    in_maps = []
    for r in range(NCORES):
        b, g = divmod(r, 4)
        rs = slice(QK * g, QK * (g + 1))
        wq = W_qkv[0:D][rs]
        wk = W_qkv[D:2 * D][rs]
        wv = W_qkv[2 * D:3 * D][rs]
        wqkT = np.ascontiguousarray(np.concatenate([wq, wk], axis=0).T).astype(np.float16)
        wvT = np.ascontiguousarray(wv.T).astype(np.float16)
        xT = np.ascontiguousarray(x[b].T).astype(np.float16)
        in_maps.append({"xT": xT, "wqkT": wqkT, "wvT": wvT,
                        "wpT": wpT, "triu": triu})
    return in_maps


def assemble(results):
    y = np.empty((B, T, D), dtype=np.float32)
    for r in range(NCORES):
        yr = results[r]["y"]
        for b in range(B):
            y[b, r * TBLK:(r + 1) * TBLK, :] = yr[b]
    return y


def kernel(**inputs):
    x = np.asarray(inputs["x"], dtype=np.float32)
    W_qkv = np.asarray(inputs["W_qkv"], dtype=np.float32)
    W_proj = np.asarray(inputs["W_proj"], dtype=np.float32)
    nc = _get_nc()
    in_maps = make_in_maps(x, W_qkv, W_proj)
    res = run_bass_kernel_spmd(nc, in_maps, core_ids=list(range(NCORES)))
    return assemble(res.results)

